# revision 7
# baseline (speedup 1.0000x reference)
"""Trainium2 Bass kernel for the SCON linear-SDE particle scan.

Reference computation: x_{t+1} = (I + DT*W_{t+1}) x_t + DT*b_{t+1} + ds*eps_t
over 10000 steps for B=512 particles with a 3-dim state, observed every 50
steps through a [4,3] projection -> loc_y [512, 201, 4].

The transition matrices depend only on theta (14 scalars), so the whole scan
is a linear map of (x0, eps).  On the host (float64) we precompute hierarchical
propagator weights that turn the scan into three levels of PE matmuls over the
noise tensor:

  level A: chunks of 10 steps   U10[c]  = sum_t  S10[c,t] ds eps_t   (1000)
  level B: windows of 50 steps  U50[w]  = sum_g  S50[w,g] U10[5w+g]  (200)
  level C: obs propagation + projection + x0/deterministic affine part

B is sharded 64 particles per core across 8 cores (pure data parallel).
Per-core device work: stream 7.68 MB of noise (as lhsT-ready [120, 250*64]
tiles), 250 + 80 + 12 matmuls, write [64, 804] output.
"""

import numpy as np

# ---------------------------------------------------------------- constants
T_TOT = 1000.0
DT = 0.1
N = 10001
TEMP_REF = 283.0
TEMP_RISE = 5.0
GAS_R = 0.008314
NSTEP = N - 1            # 10000
B = 512
NCORE = 8
BC = B // NCORE          # 64 particles per core

L1 = 10                  # level-A chunk length (steps)
NC1 = NSTEP // L1        # 1000 chunks
CPW = 5                  # chunks per window
NW = NC1 // CPW          # 200 windows
NOBS = NW + 1            # 201 observations
OBS_EVERY = 50

SUPER = 4                # chunks per level-A matmul
NMM_A = NC1 // SUPER     # 250 level-A matmuls
KC = 3 * L1 * SUPER      # 120 contraction rows per level-A matmul
NSUP_COL = 250           # eps columns groups (one per matmul)
NTILE_A = (NMM_A + 3) // 4   # 63 psum tiles (4 matmuls/tile, last has 2)

WPS = 10                 # windows per level-B slot (30 rows of 32)
NSLOT_B = NW // WPS      # 20 slots
NTILE_B = NSLOT_B // 4   # 5 u50 tiles
TAUS_PER_SLOT = 4        # u10 tiles touched per level-B slot

NOUT = 4 * NOBS          # 804
NH = NOUT // 2           # 402  (psum free-dim per matmul)

_program_cache = None
_last_results = None     # BassKernelResults of the most recent run (for test.py)


# ------------------------------------------------------------- host math
def _forcings():
    times = np.linspace(0.0, T_TOT, N)
    temp = (TEMP_REF + TEMP_RISE * times / (80 * 24 * 365)
            + 10 * np.sin(2 * np.pi / 24 * times)
            + 10 * np.sin(2 * np.pi / (24 * 365) * times))
    I_S = 0.001 + 0.0005 * np.sin(2 * np.pi / (24 * 365) * times)
    I_D = 0.0001 + 5e-05 * np.sin(2 * np.pi / (24 * 365) * times)
    return temp, I_S, I_D


def _precompute(theta):
    """float64 propagator weights, packed into the device operand layouts."""
    theta = np.asarray(theta, np.float64)
    (kSr, kDr, kMr, EaS, EaD, EaM, aSD, aDS, aM, aMSC, uM, cS, cD, cM) = theta
    temp, I_S, I_D = _forcings()
    arr = lambda p, Ea: p * np.exp(-Ea / GAS_R * (1.0 / temp - 1.0 / TEMP_REF))
    k_S, k_D, k_M = arr(kSr, EaS), arr(kDr, EaD), arr(kMr, EaM)

    zeros = np.zeros(N)
    A0 = np.stack([-k_S, aDS * k_D, aM * aMSC * k_M])
    A1 = np.stack([aSD * k_S, -(uM + k_D), aM * (1 - aMSC) * k_M])
    A2 = np.stack([zeros, np.full(N, uM), -k_M])
    W = np.stack([A0, A1, A2]).transpose(2, 0, 1)          # [N,3,3]
    bias = np.stack([I_S, I_D, zeros], axis=1)             # [N,3]

    beta = np.clip(np.array([cS, cD, cM]), 1e-6, None)
    ds = np.sqrt(beta * DT)

    M = np.eye(3)[None] + DT * W[1:]                       # [10000,3,3]
    c = DT * bias[1:]                                      # [10000,3]

    # level A: within-chunk suffix products S10[c,tau] = M_{end}...M_{tau+1}
    Mc = M.reshape(NC1, L1, 3, 3)
    S10 = np.empty((NC1, L1, 3, 3))
    A10 = np.empty((NC1, 3, 3))
    for cI in range(NC1):
        acc = np.eye(3)
        S10[cI, L1 - 1] = acc
        for tau in range(L1 - 2, -1, -1):
            acc = acc @ Mc[cI, tau + 1]
            S10[cI, tau] = acc
        A10[cI] = S10[cI, 0] @ Mc[cI, 0]
    Gmat = (S10 * ds[None, None, None, :]).transpose(0, 1, 3, 2).reshape(NC1, 30, 3)

    # level B: within-window suffix products over chunks
    A10w = A10.reshape(NW, CPW, 3, 3)
    S50 = np.empty((NW, CPW, 3, 3))
    A50 = np.empty((NW, 3, 3))
    for w in range(NW):
        acc = np.eye(3)
        S50[w, CPW - 1] = acc
        for g in range(CPW - 2, -1, -1):
            acc = acc @ A10w[w, g + 1]
            S50[w, g] = acc
        A50[w] = S50[w, 0] @ A10w[w, 0]
    Hmat = S50.transpose(0, 1, 3, 2).reshape(NW, 3 * CPW, 3)   # [w, 3g+j, i]

    # deterministic trajectory at obs points (exact, float64)
    xd = np.zeros(3)
    detx = np.zeros((NOBS, 3))
    for t in range(NSTEP):
        xd = M[t] @ xd + c[t]
        if (t + 1) % OBS_EVERY == 0:
            detx[(t + 1) // OBS_EVERY] = xd

    # observation weights
    sub = np.arange(NOBS) * OBS_EVERY
    C1 = np.stack([(1 - aSD) * k_S[sub], (1 - aDS) * k_D[sub], (1 - aM) * k_M[sub]],
                  axis=1)
    Wobs = np.concatenate([np.broadcast_to(np.eye(3), (NOBS, 3, 3)),
                           C1[:, None, :]], axis=1)        # [NOBS,4,3]

    # level C: Rmat[(w,j),(n,o)] = sum_i Wobs[n,o,i] PhiW[n,w+1][i,j] (w < n)
    Rmat = np.zeros((3 * NW, NOUT))
    RX = np.zeros((3, NOUT))
    base = np.zeros(NOUT)
    for n in range(NOBS):
        WP = Wobs[n]
        base[4 * n:4 * n + 4] = WP @ detx[n]
        acc = WP.copy()
        for w in range(n - 1, -1, -1):
            Rmat[3 * w:3 * w + 3, 4 * n:4 * n + 4] = acc.T
            acc = acc @ A50[w]
        RX[:, 4 * n:4 * n + 4] = acc.T

    # ---------------- pack into device layouts (float32) ----------------
    # Gsb [120, 250*12]: level-A lhsT tiles, block-diag of 4 chunks
    Gsb = np.zeros((KC, NMM_A * 12), np.float32)
    for s in range(NMM_A):
        for g in range(SUPER):
            Gsb[30 * g:30 * (g + 1), 12 * s + 3 * g:12 * s + 3 * g + 3] = \
                Gmat[SUPER * s + g]

    # u10 row map: chunk c10, comp i -> row 32*((c10//4)%4) + 3*(c10%4) + i,
    #                                   col 64*(c10//16) + b
    # HB [128, 80*30]: level-B lhsT tiles; matmul mB = 4*slot + (tau - tau0)
    HB = np.zeros((128, NSLOT_B * TAUS_PER_SLOT * 30), np.float32)
    mB = 0
    for om in range(NSLOT_B):
        tau0 = (50 * om) // 16
        for tau in range(tau0, tau0 + TAUS_PER_SLOT):
            blk = HB[:, 30 * mB:30 * (mB + 1)]
            for rho in range(128):
                q = rho % 32
                if q >= 12:
                    continue
                c10 = 16 * tau + 4 * (rho // 32) + q // 3
                jj = q % 3
                if c10 >= NC1:
                    continue
                w = c10 // 5
                if w // WPS != om:
                    continue
                m = w - WPS * om
                g = c10 - CPW * w
                blk[rho, 3 * m:3 * m + 3] = Hmat[w, 3 * g + jj, :]
            mB += 1

    # u50 row map: window w, comp j -> row 32*((w//10)%4) + 3*(w%10) + j,
    #                                  col 64*(w//40) + b
    # Rsb [128, 5*804]
    Rsb = np.zeros((128, NTILE_B * NOUT), np.float32)
    for wt in range(NTILE_B):
        for rho in range(128):
            q = rho % 32
            if q >= 30:
                continue
            w = WPS * (4 * wt + rho // 32) + q // 3
            j = q % 3
            Rsb[rho, NOUT * wt:NOUT * (wt + 1)] = Rmat[3 * w + j, :]

    RXaug = np.concatenate([RX, base[None]], axis=0).astype(np.float32)  # [4,804]
    return dict(Gsb=Gsb, HB=HB, Rsb=Rsb, RXaug=RXaug)


def _pack_eps(noise_core):
    """[64,10000,3] f32 -> [120, 250*64]: row r, col 64*s+b = eps[b,t,j],
    3t+j = 120*s + r."""
    a = noise_core.reshape(BC, NSTEP * 3).T          # [30000, 64] view
    a = np.ascontiguousarray(a).reshape(NSUP_COL, KC, BC).transpose(1, 0, 2)
    return np.ascontiguousarray(a).reshape(KC, NSUP_COL * BC)


# ------------------------------------------------------------ bass program
def _build_program():
    import concourse.bass as bass
    import concourse.tile as tile
    from concourse import bacc, mybir

    f32 = mybir.dt.float32
    nc = bacc.Bacc(None, target_bir_lowering=False)

    eps_d = nc.dram_tensor("eps", [KC, NSUP_COL * BC], f32, kind="ExternalInput")
    gsb_d = nc.dram_tensor("gsb", [KC, NMM_A * 12], f32, kind="ExternalInput")
    hb_d = nc.dram_tensor("hb", [128, NSLOT_B * TAUS_PER_SLOT * 30], f32,
                          kind="ExternalInput")
    rsb_d = nc.dram_tensor("rsb", [128, NTILE_B * NOUT], f32, kind="ExternalInput")
    x0_d = nc.dram_tensor("x0aug", [4, BC], f32, kind="ExternalInput")
    rx_d = nc.dram_tensor("rxaug", [4, NOUT], f32, kind="ExternalInput")
    out_d = nc.dram_tensor("out", [BC, NOUT], f32, kind="ExternalOutput")

    NSLICE = 10
    SLW = NSUP_COL * BC // NSLICE                   # 1600 cols per eps slice
    MM_PER_SLICE = NMM_A // NSLICE                  # 25

    with tile.TileContext(nc) as tc:
        with (
            tc.tile_pool(name="consts", bufs=1) as consts,
            tc.tile_pool(name="epsp", bufs=1) as epsp,
            tc.tile_pool(name="psA", bufs=3, space="PSUM") as psA,
            tc.tile_pool(name="psB", bufs=2, space="PSUM") as psB,
            tc.tile_pool(name="psC", bufs=2, space="PSUM") as psC,
        ):
            gsb = consts.tile([KC, NMM_A * 12], f32)
            hb = consts.tile([128, NSLOT_B * TAUS_PER_SLOT * 30], f32)
            rsb = consts.tile([128, NTILE_B * NOUT], f32)
            x0t = consts.tile([4, BC], f32)
            rxt = consts.tile([4, NOUT], f32)
            u10 = consts.tile([128, NTILE_A * BC], f32)
            u50 = consts.tile([128, NTILE_B * BC], f32)
            outsb = consts.tile([BC, NOUT], f32)

            nc.sync.dma_start(out=gsb, in_=gsb_d[:])
            nc.sync.dma_start(out=hb, in_=hb_d[:])
            eps_t = []
            for i in range(NSLICE):
                et = epsp.tile([KC, SLW], f32, tag=f"eps{i}")
                nc.sync.dma_start(out=et, in_=eps_d[:, SLW * i:SLW * (i + 1)])
                eps_t.append(et)
            nc.sync.dma_start(out=x0t, in_=x0_d[:])
            nc.sync.dma_start(out=rxt, in_=rx_d[:])
            nc.sync.dma_start(out=rsb, in_=rsb_d[:])

            # ---- level A: 250 matmuls -> u10 ----
            for sgA in range(NTILE_A):
                pa = psA.tile([128, BC], f32, tag="pa")
                # matmuls only write 12 of each 32-row slot; zero the tile so
                # the evacuating copy never reads stale/garbage PSUM rows
                nc.vector.memset(pa, 0.0)
                nmm = 4 if sgA < NTILE_A - 1 else NMM_A - 4 * (NTILE_A - 1)
                for sig in range(nmm):
                    s = 4 * sgA + sig
                    nc.tensor.matmul(
                        pa[32 * sig:32 * sig + 12, :],
                        gsb[:, 12 * s:12 * (s + 1)],
                        eps_t[s // MM_PER_SLICE][:, BC * (s % MM_PER_SLICE):
                                                 BC * (s % MM_PER_SLICE) + BC],
                        start=True, stop=True, tile_position=(0, 32 * sig),
                        # sim's group checker mis-maps partition-offset outs
                        skip_group_check=(sig != 0))
                nc.vector.tensor_copy(u10[:, BC * sgA:BC * (sgA + 1)], pa)

            # ---- level B: 80 matmuls -> u50 ----
            mB = 0
            for wt in range(NTILE_B):
                pb = psB.tile([128, BC], f32, tag="pb")
                nc.vector.memset(pb, 0.0)
                for sb in range(4):
                    om = 4 * wt + sb
                    tau0 = (50 * om) // 16
                    for ti in range(TAUS_PER_SLOT):
                        tau = tau0 + ti
                        nc.tensor.matmul(
                            pb[32 * sb:32 * sb + 30, :],
                            hb[:, 30 * mB:30 * (mB + 1)],
                            u10[:, BC * tau:BC * (tau + 1)],
                            start=(ti == 0), stop=(ti == TAUS_PER_SLOT - 1),
                            tile_position=(0, 32 * sb),
                            skip_group_check=(sb != 0))
                        mB += 1
                nc.vector.tensor_copy(u50[:, BC * wt:BC * (wt + 1)], pb)

            # ---- level C: x0/det affine part + noise propagation ----
            for h in range(2):
                pc = psC.tile([BC, NH], f32, tag="pc")
                nc.tensor.matmul(pc, x0t, rxt[:, NH * h:NH * (h + 1)],
                                 start=True, stop=False)
                for wt in range(NTILE_B):
                    nc.tensor.matmul(
                        pc,
                        u50[:, BC * wt:BC * (wt + 1)],
                        rsb[:, NOUT * wt + NH * h:NOUT * wt + NH * h + NH],
                        start=False, stop=(wt == NTILE_B - 1))
                nc.vector.tensor_copy(outsb[:, NH * h:NH * (h + 1)], pc)

            nc.sync.dma_start(out=out_d[:], in_=outsb)

    nc.finalize()
    return nc


# ------------------------------------------------------------------ kernel
def kernel(theta, x0, noise, obs_every):
    global _program_cache, _last_results
    from concourse.bass_utils import run_bass_kernel_spmd

    assert int(obs_every) == OBS_EVERY
    theta = np.asarray(theta, np.float32)
    x0 = np.asarray(x0, np.float32)
    noise = np.asarray(noise, np.float32)

    ops = _precompute(theta.astype(np.float64))

    if _program_cache is None:
        _program_cache = _build_program()
    nc = _program_cache

    in_maps = []
    for q in range(NCORE):
        sl = slice(BC * q, BC * (q + 1))
        x0aug = np.concatenate([np.ascontiguousarray(x0[sl].T),
                                np.ones((1, BC), np.float32)], axis=0)
        in_maps.append({
            "eps": _pack_eps(noise[sl]),
            "gsb": ops["Gsb"],
            "hb": ops["HB"],
            "rsb": ops["Rsb"],
            "x0aug": x0aug,
            "rxaug": ops["RXaug"],
        })

    import os
    trace = bool(os.environ.get("KERNEL_TRACE"))
    res = run_bass_kernel_spmd(nc, in_maps, core_ids=list(range(NCORE)),
                               trace=trace)
    _last_results = res
    out = np.concatenate(
        [res.results[q]["out"].reshape(BC, NOBS, 4) for q in range(NCORE)], axis=0)
    return out.astype(np.float32)
